# revision 3
# baseline (speedup 1.0000x reference)
"""GQA attention kernel for Trainium2: B=1, S=4096, D=1024, H=8 heads (hd=128).

Sharding: one head per NeuronCore (8 cores). Each core computes its head's
Q/K/V projections from the full hidden states, then causal flash-style
attention on-chip, writing its [S, hd] context slice (fp16, host upcasts).

Per-core design (all matmul operands fp16, PSUM fp32):
  - host pre-lays-out x^T and W^T so all DMAs are wide and contiguous
  - QT/KT computed as [hd=128(part), S]; V^T transposed to natural [k, hd]
    blocks on the PE; PSUM->SBUF copies run on the ACT engine (idle in ph.1)
  - scores^T tiles [k=128(part), q<=512] = matmul(lhsT=KT block, rhs=QT chunk);
    diagonal tiles only compute the live [128, 512-128j] slice; causal mask is
    one shared [128,128] upper-tri multiplied over the 128-col triangle only
  - exp on ACT (scale folded in); softmax denominator Z accumulated from exp
    tiles on the DVE (2 interleaved fp16 accumulators), then one ones-matmul
    per q-chunk replicates Z across partitions (removes the per-tile Z matmul
    the PE otherwise pays)
  - ctx^T accumulated over k-blocks in PSUM; normalize by 1/Z on DVE; PE
    transposes ctx^T back to [q, hd]; single strided DMA per q-chunk
"""

import os
from contextlib import ExitStack

import numpy as np

B, S, D = 1, 4096, 1024
H = 8
HD = D // H  # 128
P = 128
QC = 512  # q-chunk (columns per scores tile)
NDC = D // P  # 8 d-chunks
NQC = S // QC  # 8 q-chunks
NKB = S // P  # 32 k-blocks
SCALE = 1.0 / float(np.sqrt(HD))


def _build_program():
    nc = _build_program_inner()
    nc.finalize()
    return nc


def _build_program_inner():
    from concourse import bacc, mybir, tile
    from concourse.masks import make_identity

    f32 = mybir.dt.float32
    f16 = mybir.dt.float16

    nc = bacc.Bacc("TRN2", target_bir_lowering=False, debug=True)

    # xh[p, n*4096 + d*512 + c] = x[512n + c, 128d + p]
    xh = nc.dram_tensor("xh", [P, S * NDC], f16, kind="ExternalInput")
    # w*[p, d*128 + o] = W[128h + o, 128d + p] for this core's head h
    wq = nc.dram_tensor("wq", [P, D], f16, kind="ExternalInput")
    wk = nc.dram_tensor("wk", [P, D], f16, kind="ExternalInput")
    wv = nc.dram_tensor("wv", [P, D], f16, kind="ExternalInput")
    # tri[r, c] = 1.0 if c >= r else 0.0 (upper triangular incl. diagonal)
    tri = nc.dram_tensor("tri", [P, P], f16, kind="ExternalInput")
    out = nc.dram_tensor("out", [S, HD], f16, kind="ExternalOutput")

    Exp = mybir.ActivationFunctionType.Exp
    Copy = mybir.ActivationFunctionType.Copy

    with ExitStack() as stack:
        tc = stack.enter_context(tile.TileContext(nc))
        constp = stack.enter_context(tc.tile_pool(name="const", bufs=1))
        qkvp = stack.enter_context(tc.tile_pool(name="qkv", bufs=1))
        xp = stack.enter_context(tc.tile_pool(name="x", bufs=1))

        ident = constp.tile([P, P], f16, tag="ident")
        make_identity(nc, ident[:])
        ones_sq = constp.tile([P, P], f16, tag="ones_sq")
        nc.gpsimd.memset(ones_sq[:], 1.0)
        tri_sb = constp.tile([P, P], f16, tag="tri")
        nc.sync.dma_start(out=tri_sb[:], in_=tri[:, :])

        qt_sb = qkvp.tile([P, S], f16, tag="qt")
        kt_sb = qkvp.tile([P, S], f16, tag="kt")
        vn_sb = qkvp.tile([P, S], f16, tag="vn")  # V natural: 32 blocks [128k,128hd]

        xsb = xp.tile([P, S * NDC], f16, tag="xsb")

        # ---------------- Phase 1: QKV projections ----------------
        with tc.tile_pool(name="w", bufs=1) as wp, \
             tc.tile_pool(name="vt", bufs=1) as vtp, \
             tc.tile_pool(name="pp1", bufs=3, space="PSUM") as pp1, \
             tc.tile_pool(name="ppt", bufs=2, space="PSUM") as ppt:
            w_sb = {}
            for name, dram in (("q", wq), ("k", wk), ("v", wv)):
                w_sb[name] = wp.tile([P, D], f16, tag=f"w{name}", name=f"w{name}")
                nc.sync.dma_start(out=w_sb[name][:], in_=dram[:, :])
            for n in range(NQC):
                nc.sync.dma_start(
                    out=xsb[:, n * (NDC * QC):(n + 1) * (NDC * QC)],
                    in_=xh[:, n * (NDC * QC):(n + 1) * (NDC * QC)],
                )

            vt_sb = vtp.tile([P, S], f16, tag="vt")

            def vtrans(n):
                # V^T chunk n -> V natural blocks via PE transpose
                for kb in range(4 * n, 4 * n + 4):
                    pt = ppt.tile([P, P], f16, tag="vtp")
                    nc.tensor.transpose(
                        out=pt[:], in_=vt_sb[:, kb * P:(kb + 1) * P], identity=ident[:]
                    )
                    nc.scalar.activation(
                        out=vn_sb[:, kb * P:(kb + 1) * P], in_=pt[:], func=Copy
                    )

            for n in range(NQC):  # 512-col chunks of S
                xbase = n * (NDC * QC)
                for name, dst in (("q", qt_sb), ("k", kt_sb), ("v", vt_sb)):
                    ps = pp1.tile([P, QC], f32, tag="qkvps")
                    for d in range(NDC):
                        nc.tensor.matmul(
                            out=ps[:],
                            lhsT=w_sb[name][:, d * HD:(d + 1) * HD],
                            rhs=xsb[:, xbase + d * QC:xbase + (d + 1) * QC],
                            start=(d == 0),
                            stop=(d == NDC - 1),
                        )
                    nc.scalar.activation(
                        out=dst[:, n * QC:(n + 1) * QC], in_=ps[:], func=Copy
                    )
                if n >= 1:
                    vtrans(n - 1)
            vtrans(NQC - 1)

        # ---------------- Phase 2: attention ----------------
        with tc.tile_pool(name="expp", bufs=34) as expp, \
             tc.tile_pool(name="accp", bufs=4) as accp, \
             tc.tile_pool(name="fin", bufs=2) as finp, \
             tc.tile_pool(name="outp", bufs=2) as outp, \
             tc.tile_pool(name="ps_s", bufs=3, space="PSUM") as ps_s, \
             tc.tile_pool(name="ps_c", bufs=2, space="PSUM") as ps_c, \
             tc.tile_pool(name="ps_z", bufs=1, space="PSUM") as ps_z, \
             tc.tile_pool(name="ps_t", bufs=2, space="PSUM") as ps_t:
            for qc in range(NQC):
                nkb = 4 * qc + 4  # causal: k-blocks 0..4qc+3
                qbase = qc * QC

                # col0 of the live region within the q-chunk for tile ki
                def c0(ki):
                    return P * (ki - 4 * qc) if ki >= 4 * qc else 0

                exps = []
                acc0 = accp.tile([P, QC], f16, tag="acc", name="acc0")
                acc1 = (
                    accp.tile([P, QC], f16, tag="acc", name="acc1")
                    if qc >= 1 else None
                )
                for ki in range(nkb):
                    lo = c0(ki)
                    s_ps = ps_s.tile([P, QC], f32, tag="sps")
                    nc.tensor.matmul(
                        out=s_ps[:, lo:],
                        lhsT=kt_sb[:, ki * P:(ki + 1) * P],
                        rhs=qt_sb[:, qbase + lo:qbase + QC],
                        start=True,
                        stop=True,
                    )
                    e_sb = expp.tile([P, QC], f16, tag="exp")
                    nc.scalar.activation(
                        out=e_sb[:, lo:], in_=s_ps[:, lo:], func=Exp, scale=SCALE
                    )
                    if ki >= 4 * qc:  # diagonal tile: mask the 128-col triangle
                        nc.vector.tensor_mul(
                            out=e_sb[:, lo:lo + P], in0=e_sb[:, lo:lo + P],
                            in1=tri_sb[:],
                        )
                    # Z accumulation on DVE (2 interleaved fp16 accumulators)
                    if ki == 0:
                        nc.vector.tensor_copy(out=acc0[:], in_=e_sb[:])
                    elif ki == 1 and qc >= 1:
                        nc.vector.tensor_copy(out=acc1[:], in_=e_sb[:])
                    else:
                        t = acc1 if (qc >= 1 and ki % 2 == 1) else acc0
                        nc.vector.tensor_add(
                            out=t[:, lo:], in0=t[:, lo:], in1=e_sb[:, lo:]
                        )
                    exps.append(e_sb)

                c_ps = ps_c.tile([P, QC], f32, tag="cps")
                for ki in range(nkb):
                    lo = c0(ki)
                    nc.tensor.matmul(
                        out=c_ps[:, lo:],
                        lhsT=vn_sb[:, ki * P:(ki + 1) * P],
                        rhs=exps[ki][:, lo:],
                        start=(ki == 0),
                        stop=(ki == nkb - 1),
                    )

                if qc >= 1:
                    nc.vector.tensor_add(out=acc0[:], in0=acc0[:], in1=acc1[:])
                z_ps = ps_z.tile([P, QC], f32, tag="zps")
                # ones lhsT replicates Z = sum_k exp across all 128 partitions
                nc.tensor.matmul(
                    out=z_ps[:], lhsT=ones_sq[:], rhs=acc0[:], start=True, stop=True
                )
                rz_sb = finp.tile([P, QC], f32, tag="rz")
                nc.vector.reciprocal(out=rz_sb[:], in_=z_ps[:])
                cs_sb = finp.tile([P, QC], f16, tag="cs")
                nc.vector.tensor_mul(out=cs_sb[:], in0=c_ps[:], in1=rz_sb[:])

                # transpose ctx^T [hd, 512] -> 4 blocks [128 q, 128 hd], DMA out
                t_ps = ps_t.tile([P, QC], f16, tag="tps")
                for qs in range(QC // P):
                    nc.tensor.transpose(
                        out=t_ps[:, qs * P:(qs + 1) * P],
                        in_=cs_sb[:, qs * P:(qs + 1) * P],
                        identity=ident[:],
                    )
                o_sb = outp.tile([P, QC], f16, tag="osb")
                nc.vector.tensor_copy(out=o_sb[:], in_=t_ps[:])
                nc.sync.dma_start(
                    out=out[qc * QC:(qc + 1) * QC, :].rearrange(
                        "(a p) h -> p a h", p=P
                    ),
                    in_=o_sb[:].rearrange("p (a h) -> p a h", h=HD),
                )

    return nc


_NC_CACHE = None


def _get_nc():
    global _NC_CACHE
    if _NC_CACHE is None:
        _NC_CACHE = _build_program()
    return _NC_CACHE


def _prep_inputs(hidden_states, Wq, Wk, Wv):
    x = np.asarray(hidden_states, dtype=np.float32).reshape(S, D)
    xh = np.ascontiguousarray(
        x.reshape(NQC, QC, NDC, P).transpose(3, 0, 2, 1).reshape(P, S * NDC)
    ).astype(np.float16)
    tri = np.triu(np.ones((P, P), dtype=np.float16))

    def wprep(W, h):
        Wh = np.asarray(W, dtype=np.float32)[h * HD:(h + 1) * HD, :]  # [o, in]
        return np.ascontiguousarray(
            Wh.reshape(HD, NDC, P).transpose(2, 1, 0).reshape(P, D)
        ).astype(np.float16)

    in_maps = []
    for h in range(H):
        in_maps.append({
            "xh": xh,
            "wq": wprep(Wq, h),
            "wk": wprep(Wk, h),
            "wv": wprep(Wv, h),
            "tri": tri,
        })
    return in_maps


def kernel(hidden_states, Wq, Wk, Wv, trace=False, **trace_kwargs):
    from concourse.bass_utils import run_bass_kernel_spmd

    in_maps = _prep_inputs(hidden_states, Wq, Wk, Wv)
    nc = _get_nc()
    res = run_bass_kernel_spmd(
        nc, in_maps, core_ids=list(range(H)), trace=trace, **trace_kwargs
    )
    ctx = np.empty((B, S, D), dtype=np.float32)
    for h in range(H):
        ctx[0, :, h * HD:(h + 1) * HD] = res.results[h]["out"].astype(np.float32)
    if trace:
        return ctx, res
    return ctx


# revision 4
# speedup vs baseline: 1.3322x; 1.3322x over previous
"""GQA attention kernel for Trainium2: B=1, S=4096, D=1024, H=8 heads (hd=128).

Sharding: one head per NeuronCore (8 cores). Each core computes its head's
Q/K/V projections from the full hidden states, then causal flash-style
attention on-chip, writing its context slice as ctx^T [hd, S] (fp16, host
transposes + upcasts).

Per-core design (fp16 matmul operands, fp32 PSUM):
  - projections run d-major in chunk-groups of 2 so each weight block is
    loaded into the PE once (explicit ldweights + ldweights=False matmuls;
    the compiler's per-matmul weight reload is disabled this way)
  - V^T -> V-natural via DMA XBAR transpose (off the PE)
  - attention processes q-chunk PAIRS: each kt/vn block is loaded once and
    used by both chunks' matmuls; score pairs land in [128,1024] PSUM tiles
    so exp runs as one wide ACT instruction
  - causal diagonal tiles only compute the live slice; mask = one shared
    [128,128] upper-tri multiply on the triangle, on GPSIMD (idle engine)
  - softmax denominator: DVE accumulates exp tiles (2 fp16 accumulators per
    q-chunk), one ones-matmul replicates Z across partitions,
    reciprocal_approx_fast (the exact DVE reciprocal costs 4 us/tile)
  - PV interleaves with scores at a lag of 2 k-blocks to keep PE dense
"""

import os
from contextlib import ExitStack

import numpy as np

B, S, D = 1, 4096, 1024
H = 8
HD = D // H  # 128
P = 128
QC = 512  # q-chunk (columns per scores tile)
NDC = D // P  # 8 d-chunks
NQC = S // QC  # 8 q-chunks
NKB = S // P  # 32 k-blocks
SCALE = 1.0 / float(np.sqrt(HD))
LAG = 2  # PV trails scores by this many k-blocks


def _build_program():
    nc = _build_program_inner()
    nc.finalize()
    return nc


def _build_program_inner():
    from concourse import bacc, mybir, tile

    f32 = mybir.dt.float32
    f16 = mybir.dt.float16

    nc = bacc.Bacc("TRN2", target_bir_lowering=False, debug=True)

    # xh[p, n*4096 + d*512 + c] = x[512n + c, 128d + p]
    xh = nc.dram_tensor("xh", [P, S * NDC], f16, kind="ExternalInput")
    # w*[p, d*128 + o] = W[128h + o, 128d + p] for this core's head h
    wq = nc.dram_tensor("wq", [P, D], f16, kind="ExternalInput")
    wk = nc.dram_tensor("wk", [P, D], f16, kind="ExternalInput")
    wv = nc.dram_tensor("wv", [P, D], f16, kind="ExternalInput")
    # tri[r, c] = 1.0 if c >= r else 0.0 (upper triangular incl. diagonal)
    tri = nc.dram_tensor("tri", [P, P], f16, kind="ExternalInput")
    out = nc.dram_tensor("out", [HD, S], f16, kind="ExternalOutput")

    Exp = mybir.ActivationFunctionType.Exp
    Copy = mybir.ActivationFunctionType.Copy

    def mm_noload(**kw):
        i = nc.tensor.matmul(**kw)
        i.ins.ldweights = False
        return i

    with ExitStack() as stack:
        tc = stack.enter_context(tile.TileContext(nc))
        constp = stack.enter_context(tc.tile_pool(name="const", bufs=1))
        qkvp = stack.enter_context(tc.tile_pool(name="qkv", bufs=1))
        xp = stack.enter_context(tc.tile_pool(name="x", bufs=1))

        ones_sq = constp.tile([P, P], f16, tag="ones_sq")
        nc.gpsimd.memset(ones_sq[:], 1.0)
        tri_sb = constp.tile([P, P], f16, tag="tri")
        nc.sync.dma_start(out=tri_sb[:], in_=tri[:, :])

        qt_sb = qkvp.tile([P, S], f16, tag="qt")
        kt_sb = qkvp.tile([P, S], f16, tag="kt")
        vn_sb = qkvp.tile([P, S], f16, tag="vn")  # V natural: 32 blocks [128k,128hd]

        xsb = xp.tile([P, S * NDC], f16, tag="xsb")

        # ---------------- Phase 1: QKV projections ----------------
        with tc.tile_pool(name="w", bufs=1) as wp, \
             tc.tile_pool(name="vt", bufs=1) as vtp, \
             tc.tile_pool(name="pp1", bufs=8, space="PSUM") as pp1:
            w_sb = {}
            for name, dram in (("q", wq), ("k", wk), ("v", wv)):
                w_sb[name] = wp.tile([P, D], f16, tag=f"w{name}", name=f"w{name}")
                nc.sync.dma_start(out=w_sb[name][:], in_=dram[:, :])
            for n in range(NQC):
                eng = nc.sync if n % 2 == 0 else nc.scalar
                eng.dma_start(
                    out=xsb[:, n * (NDC * QC):(n + 1) * (NDC * QC)],
                    in_=xh[:, n * (NDC * QC):(n + 1) * (NDC * QC)],
                )

            vt_sb = vtp.tile([P, S], f16, tag="vt")

            # d-major with one ldweights per (name, d, group of 2 chunks)
            for g in range(NQC // 2):
                for name, dst in (("k", kt_sb), ("q", qt_sb), ("v", vt_sb)):
                    ps = [
                        pp1.tile([P, QC], f32, tag="qkvps", name=f"ps{g}{name}{j}")
                        for j in range(2)
                    ]
                    for d in range(NDC):
                        nc.tensor.ldweights(w_sb[name][:, d * HD:(d + 1) * HD])
                        for j in range(2):
                            n = 2 * g + j
                            xbase = n * (NDC * QC)
                            mm_noload(
                                out=ps[j][:],
                                lhsT=w_sb[name][:, d * HD:(d + 1) * HD],
                                rhs=xsb[:, xbase + d * QC:xbase + (d + 1) * QC],
                                start=(d == 0),
                                stop=(d == NDC - 1),
                            )
                    for j in range(2):
                        n = 2 * g + j
                        nc.scalar.activation(
                            out=dst[:, n * QC:(n + 1) * QC], in_=ps[j][:], func=Copy
                        )
                # V^T group -> V natural blocks via DMA XBAR transpose
                lo, hi = g * 2 * QC, (g + 1) * 2 * QC
                nc.scalar.dma_start_transpose(
                    out=vn_sb[:, lo:hi].rearrange("p (b c) -> p b c", c=P),
                    in_=vt_sb[:, lo:hi],
                )

        # ---------------- Phase 2: attention, q-chunk pairs ----------------
        with tc.tile_pool(name="expp", bufs=8) as expp, \
             tc.tile_pool(name="accp", bufs=8) as accp, \
             tc.tile_pool(name="fin", bufs=2) as finp, \
             tc.tile_pool(name="csp", bufs=2) as csp, \
             tc.tile_pool(name="ps_s", bufs=2, space="PSUM") as ps_s, \
             tc.tile_pool(name="ps_c", bufs=3, space="PSUM") as ps_c, \
             tc.tile_pool(name="ps_z", bufs=1, space="PSUM") as ps_z:
            for pr in range(NQC // 2):
                qA, qB = 2 * pr, 2 * pr + 1
                nkA, nkB = 4 * qA + 4, 4 * qB + 4
                baseA, baseB = qA * QC, qB * QC

                accA = [
                    accp.tile([P, QC], f16, tag="acc", name=f"accA{pr}{i}")
                    for i in range(1 if qA == 0 else 2)
                ]
                accB = [
                    accp.tile([P, QC], f16, tag="acc", name=f"accB{pr}{i}")
                    for i in range(2)
                ]
                c_psA = ps_c.tile([P, QC], f32, tag="cps", name=f"cA{pr}")
                c_psB = ps_c.tile([P, QC], f32, tag="cps", name=f"cB{pr}")

                exps = []  # (e_tile, loA or None, loB)

                def lo_of(ki, q0):
                    return P * (ki - 4 * q0) if ki >= 4 * q0 else 0

                def pv(k):
                    e, loA, loB = exps[k]
                    nc.tensor.ldweights(vn_sb[:, k * P:(k + 1) * P])
                    if loA is not None:
                        mm_noload(
                            out=c_psA[:, loA:],
                            lhsT=vn_sb[:, k * P:(k + 1) * P],
                            rhs=e[:, loA:QC],
                            start=(k == 0),
                            stop=(k == nkA - 1),
                        )
                    mm_noload(
                        out=c_psB[:, loB:],
                        lhsT=vn_sb[:, k * P:(k + 1) * P],
                        rhs=e[:, QC + loB:2 * QC] if loA is not None
                        else e[:, loB:QC],
                        start=(k == 0),
                        stop=(k == nkB - 1),
                    )

                def qc_tail(q0, acc, c_ps, base):
                    if len(acc) == 2:
                        nc.vector.tensor_add(out=acc[0][:], in0=acc[0][:], in1=acc[1][:])
                    z_ps = ps_z.tile([P, QC], f32, tag="zps", name=f"z{q0}")
                    nc.tensor.ldweights(ones_sq[:])
                    mm_noload(
                        out=z_ps[:], lhsT=ones_sq[:], rhs=acc[0][:],
                        start=True, stop=True,
                    )
                    rz = finp.tile([P, QC], f32, tag="rz", name=f"rz{q0}")
                    nc.vector.reciprocal_approx_fast(out=rz[:], in_=z_ps[:])
                    cs = csp.tile([P, QC], f16, tag="cs", name=f"cs{q0}")
                    nc.vector.tensor_mul(out=cs[:], in0=c_ps[:], in1=rz[:])
                    eng = nc.sync if q0 % 2 == 0 else nc.scalar
                    eng.dma_start(out=out[:, base:base + QC], in_=cs[:])

                for ki in range(nkB):
                    a_live = ki < nkA
                    loB = lo_of(ki, qB)
                    if a_live:
                        loA = lo_of(ki, qA)
                        s = ps_s.tile([P, 2 * QC], f32, tag="sps", name=f"s{pr}_{ki}")
                        nc.tensor.ldweights(kt_sb[:, ki * P:(ki + 1) * P])
                        mm_noload(
                            out=s[:, loA:QC],
                            lhsT=kt_sb[:, ki * P:(ki + 1) * P],
                            rhs=qt_sb[:, baseA + loA:baseA + QC],
                            start=True, stop=True,
                        )
                        mm_noload(
                            out=s[:, QC:],
                            lhsT=kt_sb[:, ki * P:(ki + 1) * P],
                            rhs=qt_sb[:, baseB:baseB + QC],
                            start=True, stop=True,
                        )
                        e = expp.tile([P, 2 * QC], f16, tag="exp", name=f"e{pr}_{ki}")
                        nc.scalar.activation(
                            out=e[:, loA:], in_=s[:, loA:], func=Exp, scale=SCALE
                        )
                        if ki >= 4 * qA:  # diagonal for A: mask the triangle
                            nc.gpsimd.tensor_mul(
                                out=e[:, loA:loA + P], in0=e[:, loA:loA + P],
                                in1=tri_sb[:],
                            )
                        # Z accumulation (A half)
                        if ki == 0:
                            nc.vector.tensor_copy(out=accA[0][:], in_=e[:, :QC])
                        elif qA == 0:
                            nc.vector.tensor_add(
                                out=accA[0][:, loA:], in0=accA[0][:, loA:],
                                in1=e[:, loA:QC],
                            )
                        elif ki == 1:
                            nc.vector.tensor_copy(out=accA[1][:], in_=e[:, :QC])
                        else:
                            t = accA[ki % 2]
                            nc.vector.tensor_add(
                                out=t[:, loA:], in0=t[:, loA:], in1=e[:, loA:QC]
                            )
                        # Z accumulation (B half)
                        if ki == 0:
                            nc.vector.tensor_copy(out=accB[0][:], in_=e[:, QC:])
                        elif ki == 1:
                            nc.vector.tensor_copy(out=accB[1][:], in_=e[:, QC:])
                        else:
                            nc.vector.tensor_add(
                                out=accB[ki % 2][:], in0=accB[ki % 2][:],
                                in1=e[:, QC:],
                            )
                        exps.append((e, loA, 0))
                    else:  # B-only diagonal tile
                        s = ps_s.tile([P, 2 * QC], f32, tag="sps", name=f"s{pr}_{ki}")
                        nc.tensor.ldweights(kt_sb[:, ki * P:(ki + 1) * P])
                        mm_noload(
                            out=s[:, loB:QC],
                            lhsT=kt_sb[:, ki * P:(ki + 1) * P],
                            rhs=qt_sb[:, baseB + loB:baseB + QC],
                            start=True, stop=True,
                        )
                        e = expp.tile([P, 2 * QC], f16, tag="exp", name=f"e{pr}_{ki}")
                        nc.scalar.activation(
                            out=e[:, loB:QC], in_=s[:, loB:QC], func=Exp, scale=SCALE
                        )
                        nc.gpsimd.tensor_mul(
                            out=e[:, loB:loB + P], in0=e[:, loB:loB + P],
                            in1=tri_sb[:],
                        )
                        nc.vector.tensor_add(
                            out=accB[ki % 2][:, loB:], in0=accB[ki % 2][:, loB:],
                            in1=e[:, loB:QC],
                        )
                        exps.append((e, None, loB))
                    if ki - LAG >= 0:
                        pv(ki - LAG)
                        if ki - LAG == nkA - 1:
                            qc_tail(qA, accA, c_psA, baseA)
                for k in range(nkB - LAG, nkB):
                    pv(k)
                    if k == nkA - 1:
                        qc_tail(qA, accA, c_psA, baseA)
                qc_tail(qB, accB, c_psB, baseB)

    return nc


_NC_CACHE = None


def _get_nc():
    global _NC_CACHE
    if _NC_CACHE is None:
        _NC_CACHE = _build_program()
    return _NC_CACHE


def _prep_inputs(hidden_states, Wq, Wk, Wv):
    x = np.asarray(hidden_states, dtype=np.float32).reshape(S, D)
    xh = np.ascontiguousarray(
        x.reshape(NQC, QC, NDC, P).transpose(3, 0, 2, 1).reshape(P, S * NDC)
    ).astype(np.float16)
    tri = np.triu(np.ones((P, P), dtype=np.float16))

    def wprep(W, h):
        Wh = np.asarray(W, dtype=np.float32)[h * HD:(h + 1) * HD, :]  # [o, in]
        return np.ascontiguousarray(
            Wh.reshape(HD, NDC, P).transpose(2, 1, 0).reshape(P, D)
        ).astype(np.float16)

    in_maps = []
    for h in range(H):
        in_maps.append({
            "xh": xh,
            "wq": wprep(Wq, h),
            "wk": wprep(Wk, h),
            "wv": wprep(Wv, h),
            "tri": tri,
        })
    return in_maps


def kernel(hidden_states, Wq, Wk, Wv, trace=False, **trace_kwargs):
    from concourse.bass_utils import run_bass_kernel_spmd

    in_maps = _prep_inputs(hidden_states, Wq, Wk, Wv)
    nc = _get_nc()
    res = run_bass_kernel_spmd(
        nc, in_maps, core_ids=list(range(H)), trace=trace, **trace_kwargs
    )
    ctx = np.empty((B, S, D), dtype=np.float32)
    for h in range(H):
        ctx[0, :, h * HD:(h + 1) * HD] = res.results[h]["out"].T.astype(np.float32)
    if trace:
        return ctx, res
    return ctx


# revision 8
# speedup vs baseline: 1.4512x; 1.0893x over previous
"""GQA attention kernel for Trainium2: B=1, S=4096, D=1024, H=8 heads (hd=128).

Sharding: one head per NeuronCore (8 cores). Each core computes its head's
Q/K/V projections from the full hidden states, then causal flash-style
attention on-chip, writing its context slice as ctx^T [hd, S] (fp16, host
transposes + upcasts).

Per-core design (fp16 matmul operands, fp32 PSUM):
  - projections run chunk-major (24 matmuls per 512-col chunk of S) so the
    first matmul only waits for one weight DMA + one x chunk
  - V^T -> V-natural via DMA XBAR transpose (off the PE)
  - attention processes q-chunk PAIRS; score pairs land in [128,1024] PSUM
    tiles so exp runs as one wide ACT instruction
  - within a pair, the younger chunk's diagonal tiles are computed FIRST so
    the serial exp->mask->accum chain is hidden under pair-tile work instead
    of sitting on the end-of-pair critical path
  - causal diagonal tiles only compute the live slice; mask = one shared
    [128,128] upper-tri multiply on the triangle, on GPSIMD (idle engine)
  - softmax denominator: DVE accumulates exp tiles (2 fp16 accumulators per
    q-chunk), one ones-matmul replicates Z across partitions,
    reciprocal_approx_fast (the exact DVE reciprocal costs 4 us/tile)
  - PV runs in k-order, trailing score-tile emission by 2, to keep PE dense
"""

import os
from contextlib import ExitStack

import numpy as np

B, S, D = 1, 4096, 1024
H = 8
HD = D // H  # 128
P = 128
QC = 512  # q-chunk (columns per scores tile)
NDC = D // P  # 8 d-chunks
NQC = S // QC  # 8 q-chunks
NKB = S // P  # 32 k-blocks
SCALE = 1.0 / float(np.sqrt(HD))
LAG = 2  # PV trails score-tile emission by this many tiles


def _build_program():
    nc = _build_program_inner()
    nc.finalize()
    return nc


def _build_program_inner():
    from concourse import bacc, mybir, tile

    f32 = mybir.dt.float32
    f16 = mybir.dt.float16

    nc = bacc.Bacc("TRN2", target_bir_lowering=False, debug=True)

    # xh[p, n*4096 + d*512 + c] = x[512n + c, 128d + p]
    xh = nc.dram_tensor("xh", [P, S * NDC], f16, kind="ExternalInput")
    # w*[p, d*128 + o] = W[128h + o, 128d + p] for this core's head h
    wq = nc.dram_tensor("wq", [P, D], f16, kind="ExternalInput")
    wk = nc.dram_tensor("wk", [P, D], f16, kind="ExternalInput")
    wv = nc.dram_tensor("wv", [P, D], f16, kind="ExternalInput")
    # tri[r, c] = 1.0 if c >= r else 0.0 (upper triangular incl. diagonal)
    tri = nc.dram_tensor("tri", [P, P], f16, kind="ExternalInput")
    out = nc.dram_tensor("out", [HD, S], f16, kind="ExternalOutput")

    Exp = mybir.ActivationFunctionType.Exp

    with ExitStack() as stack:
        tc = stack.enter_context(tile.TileContext(nc))
        constp = stack.enter_context(tc.tile_pool(name="const", bufs=1))
        qkvp = stack.enter_context(tc.tile_pool(name="qkv", bufs=1))
        xp = stack.enter_context(tc.tile_pool(name="x", bufs=1))

        ones_sq = constp.tile([P, P], f16, tag="ones_sq")
        nc.gpsimd.memset(ones_sq[:], 1.0)
        tri_sb = constp.tile([P, P], f16, tag="tri")

        qt_sb = qkvp.tile([P, S], f16, tag="qt")
        kt_sb = qkvp.tile([P, S], f16, tag="kt")
        vn_sb = qkvp.tile([P, S], f16, tag="vn")  # V natural: 32 blocks [128k,128hd]

        xsb = xp.tile([P, S * NDC], f16, tag="xsb")

        # ---------------- Phase 1: QKV projections ----------------
        with tc.tile_pool(name="w", bufs=1) as wp, \
             tc.tile_pool(name="vt", bufs=1) as vtp, \
             tc.tile_pool(name="pp1", bufs=6, space="PSUM") as pp1:
            w_sb = {}
            for name in ("q", "k", "v"):
                w_sb[name] = wp.tile([P, D], f16, tag=f"w{name}", name=f"w{name}")
            # DMA order: first matmul needs only wk + x chunk 0
            nc.sync.dma_start(out=w_sb["k"][:], in_=wk[:, :])
            CHUNK = NDC * QC
            nc.scalar.dma_start(out=xsb[:, 0:CHUNK], in_=xh[:, 0:CHUNK])
            nc.sync.dma_start(out=w_sb["q"][:], in_=wq[:, :])
            nc.scalar.dma_start(out=xsb[:, CHUNK:2 * CHUNK], in_=xh[:, CHUNK:2 * CHUNK])
            nc.sync.dma_start(out=w_sb["v"][:], in_=wv[:, :])
            nc.sync.dma_start(out=tri_sb[:], in_=tri[:, :])
            for n in range(2, NQC):
                eng = nc.sync if n % 2 == 0 else nc.scalar
                eng.dma_start(
                    out=xsb[:, n * CHUNK:(n + 1) * CHUNK],
                    in_=xh[:, n * CHUNK:(n + 1) * CHUNK],
                )

            vt_sb = vtp.tile([P, S], f16, tag="vt")

            for n in range(NQC):
                xbase = n * CHUNK
                for name, dst in (("k", kt_sb), ("q", qt_sb), ("v", vt_sb)):
                    ps = pp1.tile([P, QC], f32, tag="qkvps", name=f"ps{n}{name}")
                    for d in range(NDC):
                        nc.tensor.matmul(
                            out=ps[:],
                            lhsT=w_sb[name][:, d * HD:(d + 1) * HD],
                            rhs=xsb[:, xbase + d * QC:xbase + (d + 1) * QC],
                            start=(d == 0),
                            stop=(d == NDC - 1),
                        )
                    nc.vector.tensor_copy(
                        out=dst[:, n * QC:(n + 1) * QC], in_=ps[:]
                    )
                if n % 2 == 1:
                    # V^T 2-chunk group -> V natural blocks via DMA XBAR transpose
                    lo, hi = (n - 1) * QC, (n + 1) * QC
                    nc.sync.dma_start_transpose(
                        out=vn_sb[:, lo:hi].rearrange("p (b c) -> p b c", c=P),
                        in_=vt_sb[:, lo:hi],
                    )

        # ---------------- Phase 2: attention, q-chunk pairs ----------------
        with tc.tile_pool(name="expp", bufs=10) as expp, \
             tc.tile_pool(name="accp", bufs=8) as accp, \
             tc.tile_pool(name="fin", bufs=2) as finp, \
             tc.tile_pool(name="csp", bufs=2) as csp, \
             tc.tile_pool(name="ps_s", bufs=2, space="PSUM") as ps_s, \
             tc.tile_pool(name="ps_c", bufs=3, space="PSUM") as ps_c, \
             tc.tile_pool(name="ps_z", bufs=1, space="PSUM") as ps_z:
            for pr in range(NQC // 2):
                qA, qB = 2 * pr, 2 * pr + 1
                nkA, nkB = 4 * qA + 4, 4 * qB + 4
                baseA, baseB = qA * QC, qB * QC

                accA = [
                    accp.tile([P, QC], f16, tag="acc", name=f"accA{pr}{i}")
                    for i in range(1 if qA == 0 else 2)
                ]
                accB = [
                    accp.tile([P, QC], f16, tag="acc", name=f"accB{pr}{i}")
                    for i in range(2)
                ]
                c_psA = ps_c.tile([P, QC], f32, tag="cps", name=f"cA{pr}")
                c_psB = ps_c.tile([P, QC], f32, tag="cps", name=f"cB{pr}")

                exps = {}  # ki -> (e_tile, loA or None, loB)

                # PV accumulation runs in EMISSION order (B-diagonals first);
                # summation order is arbitrary, only start/stop flags matter:
                # A bank: first/last A-live tile = ki 0 / nkA-1 (emission order
                # of A tiles is k order). B bank: first emitted = ki nkA
                # (B-diag j=0), last emitted = ki nkA-1 (final pair tile).
                def pv(k):
                    e, loA, loB = exps[k]
                    if loA is not None:
                        nc.tensor.matmul(
                            out=c_psA[:, loA:],
                            lhsT=vn_sb[:, k * P:(k + 1) * P],
                            rhs=e[:, loA:QC],
                            start=(k == 0),
                            stop=(k == nkA - 1),
                        )
                    nc.tensor.matmul(
                        out=c_psB[:, loB:],
                        lhsT=vn_sb[:, k * P:(k + 1) * P],
                        rhs=e[:, QC + loB:2 * QC] if loA is not None
                        else e[:, loB:QC],
                        start=(k == nkA),
                        stop=(k == nkA - 1),
                    )

                def qc_tail(q0, acc, c_ps, base):
                    if len(acc) == 2:
                        nc.vector.tensor_add(out=acc[0][:], in0=acc[0][:], in1=acc[1][:])
                    z_ps = ps_z.tile([P, QC], f32, tag="zps", name=f"z{q0}")
                    nc.tensor.matmul(
                        out=z_ps[:], lhsT=ones_sq[:], rhs=acc[0][:],
                        start=True, stop=True,
                    )
                    rz = finp.tile([P, QC], f32, tag="rz", name=f"rz{q0}")
                    nc.vector.reciprocal_approx_fast(out=rz[:], in_=z_ps[:])
                    cs = csp.tile([P, QC], f16, tag="cs", name=f"cs{q0}")
                    nc.vector.tensor_mul(out=cs[:], in0=c_ps[:], in1=rz[:])
                    nc.sync.dma_start(out=out[:, base:base + QC], in_=cs[:])

                # emission order: B-only diagonal tiles FIRST (their serial
                # exp->mask->accum chains hide under pair-tile work), then the
                # pair tiles in k order. PV consumption follows emission order.
                order = list(range(nkA, nkB)) + list(range(nkA))
                for idx, ki in enumerate(order):
                    loB = P * (ki - 4 * qB) if ki >= 4 * qB else 0
                    if ki < nkA:  # pair tile (A live, B full)
                        loA = P * (ki - 4 * qA) if ki >= 4 * qA else 0
                        s = ps_s.tile([P, 2 * QC], f32, tag="sps", name=f"s{pr}_{ki}")
                        nc.tensor.matmul(
                            out=s[:, loA:QC],
                            lhsT=kt_sb[:, ki * P:(ki + 1) * P],
                            rhs=qt_sb[:, baseA + loA:baseA + QC],
                            start=True, stop=True,
                        )
                        nc.tensor.matmul(
                            out=s[:, QC:],
                            lhsT=kt_sb[:, ki * P:(ki + 1) * P],
                            rhs=qt_sb[:, baseB:baseB + QC],
                            start=True, stop=True,
                        )
                        e = expp.tile([P, 2 * QC], f16, tag="exp", name=f"e{pr}_{ki}")
                        nc.scalar.activation(
                            out=e[:, loA:], in_=s[:, loA:], func=Exp, scale=SCALE
                        )
                        if ki >= 4 * qA:  # diagonal for A: mask the triangle
                            nc.gpsimd.tensor_mul(
                                out=e[:, loA:loA + P], in0=e[:, loA:loA + P],
                                in1=tri_sb[:],
                            )
                        # Z accumulation (A half)
                        if ki == 0:
                            nc.vector.tensor_copy(out=accA[0][:], in_=e[:, :QC])
                        elif qA == 0:
                            nc.vector.tensor_add(
                                out=accA[0][:, loA:], in0=accA[0][:, loA:],
                                in1=e[:, loA:QC],
                            )
                        elif ki == 1:
                            nc.vector.tensor_copy(out=accA[1][:], in_=e[:, :QC])
                        else:
                            t = accA[ki % 2]
                            nc.vector.tensor_add(
                                out=t[:, loA:], in0=t[:, loA:], in1=e[:, loA:QC]
                            )
                        # Z accumulation (B half); parities were initialized by
                        # the B-diagonal tiles that were emitted first
                        nc.vector.tensor_add(
                            out=accB[ki % 2][:], in0=accB[ki % 2][:], in1=e[:, QC:]
                        )
                        exps[ki] = (e, loA, 0)
                    else:  # B-only diagonal tile, emitted first
                        s = ps_s.tile([P, 2 * QC], f32, tag="sps", name=f"s{pr}_{ki}")
                        nc.tensor.matmul(
                            out=s[:, loB:QC],
                            lhsT=kt_sb[:, ki * P:(ki + 1) * P],
                            rhs=qt_sb[:, baseB + loB:baseB + QC],
                            start=True, stop=True,
                        )
                        e = expp.tile([P, 2 * QC], f16, tag="exp", name=f"e{pr}_{ki}")
                        nc.scalar.activation(
                            out=e[:, loB:QC], in_=s[:, loB:QC], func=Exp, scale=SCALE
                        )
                        nc.gpsimd.tensor_mul(
                            out=e[:, loB:loB + P], in0=e[:, loB:loB + P],
                            in1=tri_sb[:],
                        )
                        par = ki % 2
                        if idx == 0:
                            nc.vector.tensor_copy(out=accB[par][:], in_=e[:, :QC])
                        elif idx == 1:
                            nc.vector.memset(accB[par][:, :loB], 0.0)
                            nc.vector.tensor_copy(
                                out=accB[par][:, loB:], in_=e[:, loB:QC]
                            )
                        else:
                            nc.vector.tensor_add(
                                out=accB[par][:, loB:], in0=accB[par][:, loB:],
                                in1=e[:, loB:QC],
                            )
                        exps[ki] = (e, None, loB)
                    # PV trails emission by LAG tiles, in emission order
                    if idx - LAG >= 0:
                        pv(order[idx - LAG])
                for k in order[len(order) - LAG:]:
                    pv(k)
                qc_tail(qA, accA, c_psA, baseA)
                qc_tail(qB, accB, c_psB, baseB)

    return nc


_NC_CACHE = None


def _get_nc():
    global _NC_CACHE
    if _NC_CACHE is None:
        _NC_CACHE = _build_program()
    return _NC_CACHE


def _prep_inputs(hidden_states, Wq, Wk, Wv):
    x = np.asarray(hidden_states, dtype=np.float32).reshape(S, D)
    xh = np.ascontiguousarray(
        x.reshape(NQC, QC, NDC, P).transpose(3, 0, 2, 1).reshape(P, S * NDC)
    ).astype(np.float16)
    tri = np.triu(np.ones((P, P), dtype=np.float16))

    def wprep(W, h):
        Wh = np.asarray(W, dtype=np.float32)[h * HD:(h + 1) * HD, :]  # [o, in]
        return np.ascontiguousarray(
            Wh.reshape(HD, NDC, P).transpose(2, 1, 0).reshape(P, D)
        ).astype(np.float16)

    in_maps = []
    for h in range(H):
        in_maps.append({
            "xh": xh,
            "wq": wprep(Wq, h),
            "wk": wprep(Wk, h),
            "wv": wprep(Wv, h),
            "tri": tri,
        })
    return in_maps


def kernel(hidden_states, Wq, Wk, Wv, trace=False, **trace_kwargs):
    from concourse.bass_utils import run_bass_kernel_spmd

    in_maps = _prep_inputs(hidden_states, Wq, Wk, Wv)
    nc = _get_nc()
    res = run_bass_kernel_spmd(
        nc, in_maps, core_ids=list(range(H)), trace=trace, **trace_kwargs
    )
    ctx = np.empty((B, S, D), dtype=np.float32)
    for h in range(H):
        ctx[0, :, h * HD:(h + 1) * HD] = res.results[h]["out"].T.astype(np.float32)
    if trace:
        return ctx, res
    return ctx
